# revision 1
# baseline (speedup 1.0000x reference)
"""Causal GQA attention block (B=2, T=2048, C=2048, H=16, HKV=4, D=128, RoPE)
on 8 Trainium2 NeuronCores.

Sharding: core c handles batch b = c//4 and kv-group g = c%4 (4 q heads +
1 kv head per core).  The output projection is row-parallel: each core
produces a partial [T, C] contribution; the host sums the 4 partials per
batch.

Device-side design:
  - x is host-transposed to xT [C, T] so projections contract over the
    partition dim without any on-device transpose.
  - q/k are produced in head-transposed layout [D, T]; RoPE is applied
    there using a host-side even/odd permutation folded into wq/wk plus
    partition-swapped multiplies (out = q*cos + swap(q)*sin).
  - scores are computed transposed (S.T tiles [s, t]) so exp'd tiles feed
    the attn@v matmul directly as the moving operand; the softmax
    denominator comes from a ones-vector matmul accumulated alongside.
  - no max-subtraction in softmax: scores here are O(6), exp is safe in
    fp32, and the result is mathematically identical to the reference.
  - jt0 projections run k-outer (consuming streamed weight chunks), later
    chunks run output-major over an SBUF-cached x chunk set with a
    6-bank rotating PSUM pool so RoPE overlaps accumulation.

Matmul precision mode (env BASS_ATTN_MODE): "f32r" (default; fp32 storage,
float32r matmuls, ~1.4e-4 matmul rel err at full speed), "f32" (exact, 4x
slower), "bf16" (bf16 storage).
"""

import os
from contextlib import ExitStack

import numpy as np

import concourse.bass as bass
import concourse.tile as tile
from concourse import bacc, mybir
from concourse.bass_utils import run_bass_kernel_spmd
from concourse.masks import make_identity

# problem constants
B, T, C = 2, 2048, 2048
H, HKV, D = 16, 4, 128
GROUP = H // HKV           # 4 q heads per kv head
THETA = 1000000.0
SCALE = D ** -0.5

P = 128                    # partitions
TCH = 512                  # t-chunk (matmul moving free dim)
NJT = T // TCH             # 4 t-chunks
NK = C // P                # 16 contraction tiles
NH = GROUP                 # 4 local q heads
NST = T // P               # 16 s-tiles
N_CORES = 8

F32 = mybir.dt.float32


def _sb_dt(mode):
    if mode == "bf16":
        return mybir.dt.bfloat16
    if mode == "f32r":
        return mybir.dt.float32r
    return F32


def _np_dt(mode):
    if mode == "bf16":
        import ml_dtypes
        return ml_dtypes.bfloat16
    return np.float32


def build_program(mode="f32r", phases="ABC", variant=""):
    """Build and compile the per-core Bass program. Returns nc.

    phases/variant are diagnostic knobs for timeline bisection; production
    callers use the defaults.
    """
    sb_dt = _sb_dt(mode)

    nc = bacc.Bacc("TRN2", target_bir_lowering=False, debug=False)

    xT_d = nc.dram_tensor("xT", [C, T], sb_dt, kind="ExternalInput").ap()
    wq_d = nc.dram_tensor("wqT", [C, NH * D], sb_dt, kind="ExternalInput").ap()
    wk_d = nc.dram_tensor("wkT", [C, D], sb_dt, kind="ExternalInput").ap()
    wv_d = nc.dram_tensor("wvT", [C, D], sb_dt, kind="ExternalInput").ap()
    wo_d = nc.dram_tensor("woT", [NH * D, C], sb_dt, kind="ExternalInput").ap()
    cos_d = nc.dram_tensor("cosT", [P, T], F32, kind="ExternalInput").ap()
    sin_d = nc.dram_tensor("sinT", [P, T], F32, kind="ExternalInput").ap()
    msk_d = nc.dram_tensor("mskT", [P, 4 * TCH], sb_dt, kind="ExternalInput").ap()
    ones_d = nc.dram_tensor("ones", [P, 1], sb_dt, kind="ExternalInput").ap()
    y_d = nc.dram_tensor("y", [T, C], F32, kind="ExternalOutput").ap()

    with tile.TileContext(nc) as tc, ExitStack() as ctx:
        wpool = ctx.enter_context(tc.tile_pool(name="weights", bufs=1))
        tpool = ctx.enter_context(tc.tile_pool(name="tables", bufs=1))
        state = ctx.enter_context(tc.tile_pool(name="state", bufs=1))

        # wq in 2-k chunks on the scalar queue (consumed in k order by the
        # k-outer jt0 projections); wk/wv whole on gpsimd
        wq_sb = wpool.tile([P, NK, NH * D], sb_dt, tag="wq")
        wk_sb = wpool.tile([P, NK, D], sb_dt, tag="wk")
        wv_sb = wpool.tile([P, NK, D], sb_dt, tag="wv")
        for k2 in range(NK // 2):
            nc.scalar.dma_start(
                wq_sb[:, 2 * k2:2 * k2 + 2, :],
                wq_d[2 * k2 * P:(2 * k2 + 2) * P, :].rearrange(
                    "(ko p) o -> p ko o", p=P))
        nc.gpsimd.dma_start(wk_sb[:], wk_d.rearrange("(ko p) o -> p ko o", p=P))
        nc.gpsimd.dma_start(wv_sb[:], wv_d.rearrange("(ko p) o -> p ko o", p=P))

        cos_sb = tpool.tile([P, T], F32, tag="cos")
        sin_sb = tpool.tile([P, T], F32, tag="sin")
        for jt in range(NJT):
            sl = slice(jt * TCH, (jt + 1) * TCH)
            nc.gpsimd.dma_start(cos_sb[:, sl], cos_d[:, sl])
            nc.gpsimd.dma_start(sin_sb[:, sl], sin_d[:, sl])
        msk_sb = tpool.tile([P, 4 * TCH], sb_dt, tag="msk")
        nc.gpsimd.dma_start(msk_sb[:], msk_d[:])
        ones_sb = tpool.tile([P, 1], sb_dt, tag="ones")
        nc.gpsimd.dma_start(ones_sb[:], ones_d[:])
        ident_sb = tpool.tile([P, P], F32, tag="ident")
        make_identity(nc, ident_sb[:])

        qrot = state.tile([P, NH, T], sb_dt, tag="qrot")
        krot = state.tile([P, T], sb_dt, tag="krot")
        v_sb = state.tile([P, NST, D], sb_dt, tag="v")
        ot_sb = state.tile([P, NH, T], sb_dt, tag="ot")

        # ---------------- Phase A: projections + RoPE -----------------
        with ExitStack() as actx:
          if "A" in phases:
            xpool = actx.enter_context(tc.tile_pool(name="xsub", bufs=1))
            xpool2 = actx.enter_context(tc.tile_pool(name="xsub2", bufs=2))
            ropep = actx.enter_context(tc.tile_pool(name="rope", bufs=2))
            vtp = actx.enter_context(tc.tile_pool(name="vt", bufs=2))
            psA = actx.enter_context(tc.tile_pool(name="psA", bufs=1, space="PSUM"))
            psT = actx.enter_context(tc.tile_pool(name="psT", bufs=2, space="PSUM"))

            def rope(acc_ps, out_ap, jt):
                ch = slice(jt * TCH, (jt + 1) * TCH)
                m1 = ropep.tile([P, TCH], F32, tag="m1")
                m2 = ropep.tile([P, TCH], F32, tag="m2")
                nc.vector.tensor_tensor(
                    m1[:], acc_ps[:], cos_sb[:, ch], mybir.AluOpType.mult)
                nc.vector.tensor_tensor(
                    m2[0:64, :], acc_ps[64:128, :], sin_sb[0:64, ch],
                    mybir.AluOpType.mult)
                nc.vector.tensor_tensor(
                    m2[64:128, :], acc_ps[0:64, :], sin_sb[64:128, ch],
                    mybir.AluOpType.mult)
                nc.vector.tensor_tensor(
                    out_ap, m1[:], m2[:], mybir.AluOpType.add)

            def w_slice(o, k):
                # output index o: 0..3 = q heads, 4 = k, 5 = v
                if o < NH:
                    return wq_sb[:, k, o * D:(o + 1) * D]
                if o == NH:
                    return wk_sb[:, k, :]
                return wv_sb[:, k, :]

            def finish(o, acc, jt):
                if o < NH:
                    rope(acc, qrot[:, o, jt * TCH:(jt + 1) * TCH], jt)
                elif o == NH:
                    rope(acc, krot[:, jt * TCH:(jt + 1) * TCH], jt)
                else:
                    vt = vtp.tile([P, TCH], F32, tag="vt")
                    nc.scalar.copy(vt[:], acc[:])
                    for i in range(TCH // P):
                        pst = psT.tile([P, P], F32, tag="pst")
                        nc.tensor.transpose(pst[:], vt[:, i * P:(i + 1) * P],
                                            ident_sb[:])
                        nc.scalar.copy(v_sb[:, jt * (TCH // P) + i, :], pst[:])

            nacc = 0  # rotating psum bank index
            for jt in range(NJT):
                xs = []
                for k in range(NK):
                    pool_k = xpool2 if k < 5 else xpool
                    xt = pool_k.tile([P, TCH], sb_dt, tag=f"x{k}")
                    nc.sync.dma_start(
                        xt[:],
                        xT_d[k * P:(k + 1) * P, jt * TCH:(jt + 1) * TCH])
                    xs.append(xt)

                if jt == 0:
                    # k-outer: consume weight chunks as they stream in
                    accs = [psA.tile([P, TCH], F32, tag=f"acc{o}", name=f"acc{o}")
                            for o in range(6)]
                    for k in range(NK):
                        for o in range(6):
                            nc.tensor.matmul(
                                accs[o][:], w_slice(o, k), xs[k][:],
                                start=(k == 0), stop=(k == NK - 1))
                    for o in (5, 4, 0, 1, 2, 3):
                        finish(o, accs[o], jt)
                    nacc = 6
                else:
                    # output-major: RoPE of one output overlaps the next
                    # output's accumulation via the rotating bank pool
                    for o in range(6):
                        acc = psA.tile([P, TCH], F32, tag=f"acc{nacc % 6}",
                                       name=f"accr{nacc % 6}")
                        nacc += 1
                        for k in range(NK):
                            nc.tensor.matmul(
                                acc[:], w_slice(o, k), xs[k][:],
                                start=(k == 0), stop=(k == NK - 1))
                        finish(o, acc, jt)

        # ---------------- Phase B: attention ---------------------------
        # prefetch the output-projection weights during attention so phase C
        # starts without a DMA stall
        wopool = ctx.enter_context(tc.tile_pool(name="wo", bufs=1))
        wo_sb = wopool.tile([P, NH, C], sb_dt, tag="wo")
        for h in range(NH):
            nc.scalar.dma_start(wo_sb[:, h, :], wo_d[h * P:(h + 1) * P, :])

        with ExitStack() as bctx:
          if "B" in phases:
            esp = bctx.enter_context(tc.tile_pool(name="es", bufs=5))
            rcp = bctx.enter_context(tc.tile_pool(name="rc", bufs=2))
            psS = bctx.enter_context(tc.tile_pool(name="psS", bufs=4, space="PSUM"))
            psO = bctx.enter_context(tc.tile_pool(name="psO", bufs=3, space="PSUM"))
            psD = bctx.enter_context(tc.tile_pool(name="psD", bufs=1, space="PSUM"))

            for jt in range(NJT):
                for h in range(NH):
                    njs = 4 * jt + 4
                    ot_ps = psO.tile([P, TCH], F32, tag="ot")
                    dn_ps = psD.tile([1, TCH], F32, tag="dn")
                    qch = qrot[:, h, jt * TCH:(jt + 1) * TCH]

                    def emit_pv(es, js, njs=njs, ot_ps=ot_ps, dn_ps=dn_ps):
                        nc.tensor.matmul(
                            ot_ps[:], v_sb[:, js, :], es[:],
                            start=(js == 0), stop=(js == njs - 1))
                        if variant != "noden":
                            nc.tensor.matmul(
                                dn_ps[:], ones_sb[:, 0:1], es[:],
                                start=(js == 0), stop=(js == njs - 1))

                    pend = []  # deferred two steps to hide exp latency
                    for js in range(njs):
                        s_ps = psS.tile([P, TCH], F32, tag="s")
                        nc.tensor.matmul(
                            s_ps[:], krot[:, js * P:(js + 1) * P], qch,
                            start=True, stop=True)
                        es = esp.tile([P, TCH], sb_dt, tag="es")
                        nc.scalar.activation(
                            es[:], s_ps[:], mybir.ActivationFunctionType.Exp)
                        if js // 4 == jt:
                            nc.vector.tensor_tensor(
                                es[:], es[:],
                                msk_sb[:, (js % 4) * TCH:(js % 4 + 1) * TCH],
                                mybir.AluOpType.mult)
                        if len(pend) >= 3:
                            emit_pv(*pend.pop(0))
                        pend.append((es, js))
                    for p_ in pend:
                        emit_pv(*p_)

                    if variant == "noden":
                        nc.vector.tensor_copy(
                            ot_sb[:, h, jt * TCH:(jt + 1) * TCH], ot_ps[:])
                    else:
                        rec = rcp.tile([1, TCH], F32, tag="rec")
                        nc.vector.reciprocal(rec[:], dn_ps[:])
                        rb = rcp.tile([P, TCH], F32, tag="rb")
                        nc.gpsimd.partition_broadcast(rb[:], rec[:])
                        nc.vector.tensor_tensor(
                            ot_sb[:, h, jt * TCH:(jt + 1) * TCH], ot_ps[:],
                            rb[:], mybir.AluOpType.mult)

        # ---------------- Phase C: output projection -------------------
        with ExitStack() as cctx:
          if "C" in phases:
            ypool = cctx.enter_context(tc.tile_pool(name="ysb", bufs=8))
            psC = cctx.enter_context(tc.tile_pool(name="psC", bufs=4, space="PSUM"))

            for tt in range(NST):
                for jc in range(NJT):
                    yp = psC.tile([P, TCH], F32, tag="y")
                    for h in range(NH):
                        nc.tensor.matmul(
                            yp[:],
                            ot_sb[:, h, tt * P:(tt + 1) * P],
                            wo_sb[:, h, jc * TCH:(jc + 1) * TCH],
                            start=(h == 0), stop=(h == NH - 1))
                    ys = ypool.tile([P, TCH], F32, tag="ys")
                    nc.vector.tensor_copy(ys[:], yp[:])
                    nc.sync.dma_start(
                        y_d[tt * P:(tt + 1) * P, jc * TCH:(jc + 1) * TCH],
                        ys[:])

    nc.compile()
    return nc


def host_prep(x, wq, wk, wv, wo, mode="f32r"):
    """Build the 8 per-core input maps (numpy, host-side reshuffles only)."""
    ndt = _np_dt(mode)
    x = np.asarray(x, dtype=np.float32)
    wq = np.asarray(wq, dtype=np.float32)
    wk = np.asarray(wk, dtype=np.float32)
    wv = np.asarray(wv, dtype=np.float32)
    wo = np.asarray(wo, dtype=np.float32)

    # RoPE even/odd grouping permutation within each head
    perm = np.concatenate([np.arange(0, D, 2), np.arange(1, D, 2)])

    # rope tables, transposed layout [d, t], matching reference f32 math
    inv_freq = (1.0 / THETA ** (np.arange(0, D, 2, dtype=np.float32) / D)).astype(np.float32)
    pos = np.arange(T, dtype=np.float32)
    freqs = pos[:, None] * inv_freq[None, :]          # [T, 64] f32
    cos_t = np.cos(freqs).astype(np.float32).T        # [64, T]
    sin_t = np.sin(freqs).astype(np.float32).T        # [64, T]
    cosT = np.concatenate([cos_t, cos_t], axis=0)     # [128, T]
    sinT = np.concatenate([-sin_t, sin_t], axis=0)    # [128, T]

    # diagonal-block causal masks (multiplicative, after exp)
    # pattern r (= js % 4): allow f >= 128*r + p
    f = np.arange(TCH)[None, :]
    p = np.arange(P)[:, None]
    msk = np.concatenate(
        [(f >= P * r + p).astype(np.float32) for r in range(4)], axis=1)

    xTs = [np.ascontiguousarray(x[b].T).astype(ndt) for b in range(B)]

    in_maps = []
    for c in range(N_CORES):
        b, g = divmod(c, GROUP)
        rows = []
        for hh in range(NH):
            h = g * GROUP + hh
            rows.append(wq[h * D + perm, :])
        wq_g = np.concatenate(rows, axis=0) * SCALE          # [512, C]
        wk_g = wk[g * D + perm, :]                           # [128, C]
        wv_g = wv[g * D:(g + 1) * D, :]                      # [128, C]
        wo_g = wo[:, g * NH * D:(g + 1) * NH * D]            # [C, 512]

        in_maps.append({
            "xT": xTs[b],
            "wqT": np.ascontiguousarray(wq_g.T).astype(ndt),
            "wkT": np.ascontiguousarray(wk_g.T).astype(ndt),
            "wvT": np.ascontiguousarray(wv_g.T).astype(ndt),
            "woT": np.ascontiguousarray(wo_g.T).astype(ndt),
            "cosT": cosT,
            "sinT": sinT,
            "mskT": msk.astype(ndt),
            "ones": np.ones((P, 1), dtype=ndt),
        })
    return in_maps


_CACHE = {}


def _get_program(mode):
    if mode not in _CACHE:
        _CACHE[mode] = build_program(mode)
    return _CACHE[mode]


def kernel(x, mask, wq, wk, wv, wo):
    mode = os.environ.get("BASS_ATTN_MODE", "f32r")
    nc = _get_program(mode)
    in_maps = host_prep(x, wq, wk, wv, wo, mode)
    res = run_bass_kernel_spmd(nc, in_maps, list(range(N_CORES))).results
    out = np.zeros((B, T, C), dtype=np.float32)
    for c in range(N_CORES):
        out[c // GROUP] += res[c]["y"]
    return out



# revision 4
# speedup vs baseline: 1.3803x; 1.3803x over previous
"""Causal GQA attention (B=2, T=2048, C=2048, H=16, HKV=4, D=128, RoPE)
on 8 Trainium2 NeuronCores — v2.

Sharding: core c = (batch b = c//4, kv-group g = c%4): 4 q heads + 1 kv
head per core; row-parallel output projection, host sums 4 partials.

Design notes:
  - Projections run as fp8e4 hi/lo DoubleRow matmuls (3 compensation
    terms over k-tile pairs). x and w ship pre-split/pre-scaled; dequant
    folds into the RoPE tables (q/k), the v-copy scale, and the softmax
    scale applied at exp time.
  - bf16 storage elsewhere; fp8 only where noise is attenuated
    (off-diagonal softmax weights + hi/lo v).
  - Attention is block-causal at 128 granularity via suffix-truncated
    moving operands on the diagonal; off-diagonal s-tile pairs exp
    straight to fp8 and feed DoubleRow PV (v hi/lo) and DoubleRow
    ones-matmul denominators.
  - Output projection (bf16) is interleaved into phase B's instruction
    stream to fill the tensor engine during Act-bound stretches.
  - DMA pacing: wo deferred to phase B, trig tables per-chunk bf16 with
    pool-rotation gating, x triple-buffered.
"""

import os
from contextlib import ExitStack

import numpy as np

import concourse.bass as bass
import concourse.tile as tile
from concourse import bacc, mybir
from concourse.bass_utils import run_bass_kernel_spmd
from concourse.masks import make_identity

B, T, C = 2, 2048, 2048
H, HKV, D = 16, 4, 128
GROUP = H // HKV
THETA = 1000000.0
SCALE = D ** -0.5

P = 128
TCH = 512
NJT = T // TCH             # 4
NK = C // P                # 16 k-tiles
NKP = NK // 2              # 8 DoubleRow k-tile pairs
NH = GROUP                 # 4 local q heads
NST = T // P               # 16 s-tiles
N_CORES = 8

SX = 16.0                  # x fp8 scale
SW = 1024.0                # w fp8 scale (q/k/v)
SV = 2.0 ** -10            # v psum -> sbuf scale
CDEN = 0.25                # ones constant: ot = 64*O_true (fp8 range)
SWO = 1024.0               # wo fp8 scale
YDQ = 1.0 / (64.0 * SWO)   # host-side dequant of the y partials

F32 = mybir.dt.float32
BF16 = mybir.dt.bfloat16
F8 = mybir.dt.float8e4
DR = mybir.MatmulPerfMode.DoubleRow


def build_program(phases="ABC", variant=""):
    nc = bacc.Bacc("TRN2", target_bir_lowering=False, debug=False)

    xh_d = nc.dram_tensor("xh", [C, T], F8, kind="ExternalInput").ap()
    xl_d = nc.dram_tensor("xl", [C, T], F8, kind="ExternalInput").ap()
    wqh_d = nc.dram_tensor("wqh", [C, NH * D], F8, kind="ExternalInput").ap()
    wql_d = nc.dram_tensor("wql", [C, NH * D], F8, kind="ExternalInput").ap()
    wkh_d = nc.dram_tensor("wkh", [C, D], F8, kind="ExternalInput").ap()
    wkl_d = nc.dram_tensor("wkl", [C, D], F8, kind="ExternalInput").ap()
    wvh_d = nc.dram_tensor("wvh", [C, D], F8, kind="ExternalInput").ap()
    wvl_d = nc.dram_tensor("wvl", [C, D], F8, kind="ExternalInput").ap()
    woh_d = nc.dram_tensor("woh", [P, 2 * NJT * 2 * TCH], F8, kind="ExternalInput").ap()
    wol_d = nc.dram_tensor("wol", [P, 2 * NJT * 2 * TCH], F8, kind="ExternalInput").ap()
    cos_d = nc.dram_tensor("cosT", [P, T], BF16, kind="ExternalInput").ap()
    sin_d = nc.dram_tensor("sinT", [P, T], BF16, kind="ExternalInput").ap()
    tri_d = nc.dram_tensor("tri", [P, P], BF16, kind="ExternalInput").ap()
    on8_d = nc.dram_tensor("on8", [P, 256], F8, kind="ExternalInput").ap()
    onb_d = nc.dram_tensor("onb", [P, P], BF16, kind="ExternalInput").ap()
    y_d = nc.dram_tensor("y", [T, C], BF16, kind="ExternalOutput").ap()

    with tile.TileContext(nc) as tc, ExitStack() as ctx:
        wpool = ctx.enter_context(tc.tile_pool(name="weights", bufs=1))
        tpool = ctx.enter_context(tc.tile_pool(name="tables", bufs=1))
        trigp = ctx.enter_context(tc.tile_pool(name="trig", bufs=2))
        state = ctx.enter_context(tc.tile_pool(name="state", bufs=1))

        wqh_sb = wpool.tile([P, NKP, 2, NH * D], F8, tag="wqh")
        wql_sb = wpool.tile([P, NKP, 2, NH * D], F8, tag="wql")
        wkh_sb = wpool.tile([P, NKP, 2, D], F8, tag="wkh")
        wkl_sb = wpool.tile([P, NKP, 2, D], F8, tag="wkl")
        wvh_sb = wpool.tile([P, NKP, 2, D], F8, tag="wvh")
        wvl_sb = wpool.tile([P, NKP, 2, D], F8, tag="wvl")

        def wload(sb, dram):
            nc.scalar.dma_start(
                sb[:], dram.rearrange("(kp two p) o -> p kp two o", p=P, two=2))

        def wload_q(sb, dram):
            for c4 in range(4):
                nc.scalar.dma_start(
                    sb[:, 2 * c4:2 * c4 + 2, :, :],
                    dram[c4 * 4 * P:(c4 + 1) * 4 * P, :].rearrange(
                        "(kp two p) o -> p kp two o", p=P, two=2))

        wload(wkh_sb, wkh_d)
        wload(wvh_sb, wvh_d)
        wload_q(wqh_sb, wqh_d)
        wload(wkl_sb, wkl_d)
        wload(wvl_sb, wvl_d)
        wload_q(wql_sb, wql_d)

        tri_sb = tpool.tile([P, P], BF16, tag="tri")
        nc.gpsimd.dma_start(tri_sb[:], tri_d[:])
        on8_sb = tpool.tile([P, 2, P], F8, tag="on8")
        nc.gpsimd.dma_start(on8_sb[:], on8_d.rearrange("p (two f) -> p two f", two=2))
        onb_sb = tpool.tile([P, P], BF16, tag="onb")
        nc.gpsimd.dma_start(onb_sb[:], onb_d[:])
        identb = tpool.tile([P, P], BF16, tag="ident")
        make_identity(nc, identb[:])
        # (variant parsed below, before use)

        # PE warmup: keep the tensor engine continuously busy from t~0 so the
        # p-state ramp completes before the first real (DMA-gated) matmuls.
        warmp = ctx.enter_context(tc.tile_pool(name="warm", bufs=1))
        warm_sb = warmp.tile([P, P], BF16, tag="w")
        nc.vector.memset(warm_sb[:], 0.0)
        nwarm = 34
        for v in variant.split("+"):
            if v.startswith("warm"):
                nwarm = int(v[4:])
        with tc.tile_pool(name="psW", bufs=1, space="PSUM") as psW:
            wp = psW.tile([P, P], F32, tag="wp")
            for i in range(nwarm):
                nc.tensor.matmul(wp[:], identb[:], warm_sb[:],
                                 start=(i == 0), stop=(i == nwarm - 1))

        qrot = state.tile([P, NH, T], BF16, tag="qrot")
        krot = state.tile([P, T], BF16, tag="krot")
        v_sb = state.tile([P, NST, D], BF16, tag="v")
        v8h = state.tile([P, NST // 2, 2, D], F8, tag="v8h")
        v8l = state.tile([P, NST // 2, 2, D], F8, tag="v8l")
        ot8h = state.tile([P, 2, NST, 2, D], F8, tag="ot8h")
        ot8l = state.tile([P, 2, NST, 2, D], F8, tag="ot8l")

        njt_lim = NJT
        for v in variant.split("+"):
            if v.startswith("njt"):
                njt_lim = int(v[3:])

        # ---------------- Phase A: projections + RoPE -----------------
        with ExitStack() as actx:
          if "A" in phases:
            xpool = actx.enter_context(tc.tile_pool(name="xsub", bufs=3))
            ropep = actx.enter_context(tc.tile_pool(name="rope", bufs=2))
            vtp = actx.enter_context(tc.tile_pool(name="vt", bufs=2))
            psA = actx.enter_context(tc.tile_pool(name="psA", bufs=1, space="PSUM"))
            psT = actx.enter_context(tc.tile_pool(name="psT", bufs=2, space="PSUM"))

            def rope(acc_ps, out_ap, cos_t, sin_t):
                m1 = ropep.tile([P, TCH], F32, tag="m1")
                m2 = ropep.tile([P, TCH], F32, tag="m2")
                nc.vector.tensor_tensor(
                    m1[:], acc_ps[:], cos_t[:], mybir.AluOpType.mult)
                nc.vector.tensor_tensor(
                    m2[0:64, :], acc_ps[64:128, :], sin_t[0:64, :],
                    mybir.AluOpType.mult)
                nc.vector.tensor_tensor(
                    m2[64:128, :], acc_ps[0:64, :], sin_t[64:128, :],
                    mybir.AluOpType.mult)
                nc.vector.tensor_tensor(
                    out_ap, m1[:], m2[:], mybir.AluOpType.add)

            def w_slice(term, o, m):
                if o < NH:
                    sb = wqh_sb if term != 2 else wql_sb
                    return sb[:, m, :, o * D:(o + 1) * D]
                if o == NH:
                    sb = wkh_sb if term != 2 else wkl_sb
                    return sb[:, m, :, :]
                sb = wvh_sb if term != 2 else wvl_sb
                return sb[:, m, :, :]

            def finish(o, acc, jt, cos_t, sin_t):
                """Returns a deferred-emission thunk (or None)."""
                if "noropeA" in variant:
                    return None
                if o < NH:
                    rope(acc, qrot[:, o, jt * TCH:(jt + 1) * TCH], cos_t, sin_t)
                    return None
                if o == NH:
                    rope(acc, krot[:, jt * TCH:(jt + 1) * TCH], cos_t, sin_t)
                    return None
                vt = vtp.tile([P, TCH], BF16, tag="vt")
                nc.scalar.activation(
                    vt[:], acc[:], mybir.ActivationFunctionType.Copy, scale=SV)

                def transposes():
                    for i in range(TCH // P):
                        s_idx = jt * (TCH // P) + i
                        pst = psT.tile([P, P], BF16, tag="pst")
                        nc.tensor.transpose(pst[:], vt[:, i * P:(i + 1) * P],
                                            identb[:])
                        nc.scalar.copy(v_sb[:, s_idx, :], pst[:])
                        nc.scalar.copy(v8h[:, s_idx // 2, s_idx % 2, :], pst[:])
                        nc.vector.tensor_tensor(
                            v8l[:, s_idx // 2, s_idx % 2, :], pst[:],
                            v8h[:, s_idx // 2, s_idx % 2, :],
                            mybir.AluOpType.subtract)
                return transposes

            nacc = 0
            deferred = None
            for jt in range(njt_lim):
                cos_t = trigp.tile([P, TCH], BF16, tag="cos")
                sin_t = trigp.tile([P, TCH], BF16, tag="sin")
                ch = slice(jt * TCH, (jt + 1) * TCH)
                nc.gpsimd.dma_start(cos_t[:], cos_d[:, ch])
                nc.gpsimd.dma_start(sin_t[:], sin_d[:, ch])

                xhs, xls = [], []
                for m in range(NKP):
                    xt = xpool.tile([P, 2, TCH], F8, tag=f"xh{m}")
                    nc.sync.dma_start(
                        xt[:],
                        xh_d[2 * m * P:(2 * m + 2) * P, ch].rearrange(
                            "(two p) t -> p two t", p=P))
                    xhs.append(xt)
                for m in range(NKP):
                    xt = xpool.tile([P, 2, TCH], F8, tag=f"xl{m}")
                    nc.sync.dma_start(
                        xt[:],
                        xl_d[2 * m * P:(2 * m + 2) * P, ch].rearrange(
                            "(two p) t -> p two t", p=P))
                    xls.append(xt)

                if jt == 0:
                    accs = [psA.tile([P, TCH], F32, tag=f"acc{o}",
                                     name=f"acc{o}")
                            for o in range(6)]
                    for ti, (xs, term) in enumerate(((xhs, 0), (xls, 1),
                                                     (xhs, 2))):
                        for m in range(NKP):
                            for o in (4, 5, 0, 1, 2, 3):
                                nc.tensor.matmul(
                                    accs[o][:], w_slice(term, o, m),
                                    xs[m][:],
                                    start=(ti == 0 and m == 0),
                                    stop=(ti == 2 and m == NKP - 1),
                                    perf_mode=DR)
                    for o in (5, 4, 0, 1, 2, 3):
                        d = finish(o, accs[o], jt, cos_t, sin_t)
                        if d is not None:
                            deferred = d
                    nacc = 5
                else:
                    for o in (4, 5, 0, 1, 2, 3):
                        acc = psA.tile([P, TCH], F32, tag=f"acc{nacc % 6}",
                                       name=f"accr{nacc % 6}")
                        nacc += 1
                        first = True
                        for xs, term in (((xhs, 0), (xls, 1), (xhs, 2))):
                            for m in range(NKP):
                                nc.tensor.matmul(
                                    acc[:], w_slice(term, o, m), xs[m][:],
                                    start=first,
                                    stop=(term == 2 and m == NKP - 1),
                                    perf_mode=DR)
                                first = False
                        if deferred is not None:
                            deferred()
                            deferred = None
                        d = finish(o, acc, jt, cos_t, sin_t)
                        if d is not None:
                            deferred = d
                if jt == njt_lim - 1 and deferred is not None:
                    deferred()
                    deferred = None

        # ---------------- Phase B + C interleaved ----------------------
        with ExitStack() as bctx:
          if "B" in phases:
            e8p = bctx.enter_context(tc.tile_pool(name="e8", bufs=4))
            edp = bctx.enter_context(tc.tile_pool(name="ed", bufs=3))
            rcp = bctx.enter_context(tc.tile_pool(name="rc", bufs=3))
            wopool = bctx.enter_context(tc.tile_pool(name="wo", bufs=1))
            ypool = bctx.enter_context(tc.tile_pool(name="ysb", bufs=6))
            psS = bctx.enter_context(tc.tile_pool(name="psS", bufs=2, space="PSUM"))
            psO = bctx.enter_context(tc.tile_pool(name="psO", bufs=1, space="PSUM"))
            psD = bctx.enter_context(tc.tile_pool(name="psD", bufs=1, space="PSUM"))
            psC = bctx.enter_context(tc.tile_pool(name="psC", bufs=2, space="PSUM"))

            woh_sb = wopool.tile([P, 2, NJT, 2, TCH], F8, tag="woh")
            wol_sb = wopool.tile([P, 2, NJT, 2, TCH], F8, tag="wol")
            nc.sync.dma_start(
                woh_sb[:], woh_d.rearrange("p (i jc two t) -> p i jc two t",
                                           i=2, jc=NJT, two=2))
            nc.sync.dma_start(
                wol_sb[:], wol_d.rearrange("p (i jc two t) -> p i jc two t",
                                           i=2, jc=NJT, two=2))

            def c_group(tt, jc):
                yp = psC.tile([P, TCH], F32, tag="y")
                first = True
                for i in range(2):
                    for lhs, rhs in ((ot8h, woh_sb), (ot8l, woh_sb),
                                     (ot8h, wol_sb)):
                        nc.tensor.matmul(
                            yp[:], lhs[:, i, tt, :, :], rhs[:, i, jc, :, :],
                            start=first, stop=(i == 1 and rhs is wol_sb),
                            perf_mode=DR, skip_group_check=True)
                        first = False
                ys = ypool.tile([P, TCH], BF16, tag="ys")
                nc.vector.tensor_copy(ys[:], yp[:])
                nc.sync.dma_start(
                    y_d[tt * P:(tt + 1) * P, jc * TCH:(jc + 1) * TCH],
                    ys[:])

            do_c = "C" in phases

            cpend = []

            def emit_c(n=1):
                for _ in range(n):
                    if cpend:
                        c_group(*cpend.pop(0))

            for jt in range(NJT):
                ch = slice(jt * TCH, (jt + 1) * TCH)
                if do_c and jt >= 1:
                    cpend = [(4 * (jt - 1) + tt4, jc)
                             for tt4 in range(4) for jc in range(NJT)]
                for h in range(NH):
                    qch = qrot[:, h, ch]
                    ot_ps = psO.tile([P, TCH], F32, tag="ot")
                    dn_ps = psD.tile([P, TCH], F32, tag="dn")
                    npair = 2 * jt

                    def sc_pair(m):
                        sps = psS.tile([P, 2 * TCH], F32, tag="su")
                        for i in range(2):
                            js = 2 * m + i
                            nc.tensor.matmul(
                                sps[:, i * TCH:(i + 1) * TCH],
                                krot[:, js * P:(js + 1) * P], qch,
                                start=True, stop=True)
                        e8 = e8p.tile([P, 2, TCH], F8, tag="e8")
                        nc.scalar.activation(
                            e8[:], sps[:], mybir.ActivationFunctionType.Exp,
                            scale=SCALE)
                        return e8

                    def pv_pair(m, e8, start):
                        nc.tensor.matmul(
                            ot_ps[:], v8h[:, m, :, :], e8[:],
                            start=start, stop=False, perf_mode=DR,
                            skip_group_check=True)
                        nc.tensor.matmul(
                            ot_ps[:], v8l[:, m, :, :], e8[:],
                            start=False, stop=False, perf_mode=DR,
                            skip_group_check=True)
                        nc.tensor.matmul(
                            dn_ps[:], on8_sb[:], e8[:],
                            start=start, stop=False, perf_mode=DR,
                            skip_group_check=True)

                    pend = []
                    for m in range(npair):
                        e8 = sc_pair(m)
                        if len(pend) >= 3:
                            mm, ee = pend.pop(0)
                            pv_pair(mm, ee, mm == 0)
                        pend.append((m, e8))

                    dsup = []
                    for half in range(2):
                        sps = psS.tile([P, 2 * TCH], F32, tag="su")
                        ed = edp.tile([P, 2 * TCH], BF16, tag="ed")
                        widths = []
                        off = 0
                        for rr in range(2):
                            r = 2 * half + rr
                            w = TCH - r * P
                            js = 4 * jt + r
                            nc.tensor.matmul(
                                sps[:, off:off + w],
                                krot[:, js * P:(js + 1) * P],
                                qch[:, r * P:],
                                start=True, stop=True)
                            widths.append((r, off, w))
                            off += w
                        nc.scalar.activation(
                            ed[:, 0:off], sps[:, 0:off],
                            mybir.ActivationFunctionType.Exp, scale=SCALE)
                        for r, off_, w in widths:
                            nc.vector.tensor_tensor(
                                ed[:, off_:off_ + P], ed[:, off_:off_ + P],
                                tri_sb[:], mybir.AluOpType.mult)
                        dsup.append((ed, widths))
                        while pend:
                            mm, ee = pend.pop(0)
                            pv_pair(mm, ee, mm == 0)

                    for ed, widths in dsup:
                        for r, off, w in widths:
                            js = 4 * jt + r
                            nc.tensor.matmul(
                                ot_ps[:, r * P:], v_sb[:, js, :],
                                ed[:, off:off + w],
                                start=(jt == 0 and r == 0), stop=(r == 3),
                                skip_group_check=True)
                            nc.tensor.matmul(
                                dn_ps[:, r * P:], onb_sb[:],
                                ed[:, off:off + w],
                                start=(jt == 0 and r == 0), stop=(r == 3),
                                skip_group_check=True)

                    emit_c(4)
                    rb = rcp.tile([P, TCH], F32, tag="rb")
                    nc.vector.reciprocal(rb[:], dn_ps[:])
                    tmp = rcp.tile([P, 4, P], F32, tag="tmp")
                    nc.vector.tensor_tensor(
                        tmp[:].rearrange("p a b -> p (a b)"), ot_ps[:], rb[:],
                        mybir.AluOpType.mult)
                    oh = ot8h[:, h // 2, 4 * jt:4 * jt + 4, h % 2, :]
                    ol = ot8l[:, h // 2, 4 * jt:4 * jt + 4, h % 2, :]
                    nc.vector.tensor_copy(oh, tmp[:])
                    nc.vector.tensor_tensor(
                        ol, tmp[:], oh, mybir.AluOpType.subtract)


            if do_c:
                for tt4 in range(4):
                    for jc in range(NJT):
                        c_group(12 + tt4, jc)

    nc.compile()
    return nc


def host_prep(x, wq, wk, wv, wo):
    import ml_dtypes
    F8np = ml_dtypes.float8_e4m3
    BFnp = ml_dtypes.bfloat16

    x = np.asarray(x, dtype=np.float32)
    wq = np.asarray(wq, dtype=np.float32)
    wk = np.asarray(wk, dtype=np.float32)
    wv = np.asarray(wv, dtype=np.float32)
    wo = np.asarray(wo, dtype=np.float32)

    perm = np.concatenate([np.arange(0, D, 2), np.arange(1, D, 2)])

    inv_freq = (1.0 / THETA ** (np.arange(0, D, 2, dtype=np.float32) / D)).astype(np.float32)
    pos = np.arange(T, dtype=np.float32)
    freqs = pos[:, None] * inv_freq[None, :]
    cos_t = np.cos(freqs).astype(np.float32).T
    sin_t = np.sin(freqs).astype(np.float32).T
    dq = np.float32(1.0 / (SX * SW))
    cosT = np.concatenate([cos_t, cos_t], axis=0) * dq
    sinT = np.concatenate([-sin_t, sin_t], axis=0) * dq

    tri = (np.arange(P)[None, :] >= np.arange(P)[:, None]).astype(BFnp)

    def hilo(a):
        h = a.astype(F8np)
        l = (a - h.astype(np.float32)).astype(F8np)
        return h, l

    xs = [np.ascontiguousarray(x[b].T) * SX for b in range(B)]
    xhl = [hilo(a) for a in xs]

    in_maps = []
    for c in range(N_CORES):
        b, g = divmod(c, GROUP)
        rows = []
        for hh in range(NH):
            h = g * GROUP + hh
            rows.append(wq[h * D + perm, :])
        wq_g = np.concatenate(rows, axis=0) * SW          # [512, C]
        wk_g = wk[g * D + perm, :] * SW
        wv_g = wv[g * D:(g + 1) * D, :] * SW
        wo_g = wo[:, g * NH * D:(g + 1) * NH * D]         # [C, 512]

        wqh, wql = hilo(np.ascontiguousarray(wq_g.T))
        wkh, wkl = hilo(np.ascontiguousarray(wk_g.T))
        wvh, wvl = hilo(np.ascontiguousarray(wv_g.T))
        # wo pair-contiguous DR layout: [d, hpair, jc, h-in-pair, tc]
        woT = np.ascontiguousarray(wo_g.T) * SWO          # [512, C]
        wo_a = woT.reshape(2, 2, D, NJT, TCH)             # [i, hh, d, jc, tc]
        wo_b = np.ascontiguousarray(np.transpose(wo_a, (2, 0, 3, 1, 4)))
        woh, wol = hilo(wo_b.reshape(P, -1))

        in_maps.append({
            "xh": xhl[b][0], "xl": xhl[b][1],
            "wqh": wqh, "wql": wql,
            "wkh": wkh, "wkl": wkl,
            "wvh": wvh, "wvl": wvl,
            "woh": woh, "wol": wol,
            "cosT": cosT.astype(BFnp),
            "sinT": sinT.astype(BFnp),
            "tri": tri,
            "on8": np.full((P, 256), CDEN, dtype=F8np),
            "onb": np.full((P, P), CDEN, dtype=BFnp),
        })
    return in_maps


_CACHE = {}


def _get_program(key="v2"):
    if key not in _CACHE:
        _CACHE[key] = build_program()
    return _CACHE[key]


def kernel(x, mask, wq, wk, wv, wo):
    nc = _get_program()
    in_maps = host_prep(x, wq, wk, wv, wo)
    res = run_bass_kernel_spmd(nc, in_maps, list(range(N_CORES))).results
    out = np.zeros((B, T, C), dtype=np.float32)
    for c in range(N_CORES):
        out[c // GROUP] += res[c]["y"].astype(np.float32) * YDQ
    return out


# revision 5
# speedup vs baseline: 1.3819x; 1.0011x over previous
"""Causal GQA attention (B=2, T=2048, C=2048, H=16, HKV=4, D=128, RoPE)
on 8 Trainium2 NeuronCores — v2.

Sharding: core c = (batch b = c//4, kv-group g = c%4): 4 q heads + 1 kv
head per core; row-parallel output projection, host sums 4 partials.

Design notes:
  - Projections run as fp8e4 hi/lo DoubleRow matmuls (3 compensation
    terms over k-tile pairs). x and w ship pre-split/pre-scaled; dequant
    folds into the RoPE tables (q/k), the v-copy scale, and the softmax
    scale applied at exp time.
  - bf16 storage elsewhere; fp8 only where noise is attenuated
    (off-diagonal softmax weights + hi/lo v).
  - Attention is block-causal at 128 granularity via suffix-truncated
    moving operands on the diagonal; off-diagonal s-tile pairs exp
    straight to fp8 and feed DoubleRow PV (v hi/lo) and DoubleRow
    ones-matmul denominators.
  - Output projection (bf16) is interleaved into phase B's instruction
    stream to fill the tensor engine during Act-bound stretches.
  - DMA pacing: wo deferred to phase B, trig tables per-chunk bf16 with
    pool-rotation gating, x triple-buffered.
"""

import os
from contextlib import ExitStack

import numpy as np

import concourse.bass as bass
import concourse.tile as tile
from concourse import bacc, mybir
from concourse.bass_utils import run_bass_kernel_spmd
from concourse.masks import make_identity

B, T, C = 2, 2048, 2048
H, HKV, D = 16, 4, 128
GROUP = H // HKV
THETA = 1000000.0
SCALE = D ** -0.5

P = 128
TCH = 512
NJT = T // TCH             # 4
NK = C // P                # 16 k-tiles
NKP = NK // 2              # 8 DoubleRow k-tile pairs
NH = GROUP                 # 4 local q heads
NST = T // P               # 16 s-tiles
N_CORES = 8

SX = 16.0                  # x fp8 scale
SW = 1024.0                # w fp8 scale (q/k/v)
SV = 2.0 ** -10            # v psum -> sbuf scale
CDEN = 0.25                # ones constant: ot = 64*O_true (fp8 range)
SWO = 1024.0               # wo fp8 scale
YDQ = 1.0 / (64.0 * SWO)   # host-side dequant of the y partials

F32 = mybir.dt.float32
BF16 = mybir.dt.bfloat16
F8 = mybir.dt.float8e4
DR = mybir.MatmulPerfMode.DoubleRow


def build_program(phases="ABC", variant=""):
    nc = bacc.Bacc("TRN2", target_bir_lowering=False, debug=False)

    xh_d = nc.dram_tensor("xh", [C, T], F8, kind="ExternalInput").ap()
    xl_d = nc.dram_tensor("xl", [C, T], F8, kind="ExternalInput").ap()
    wqh_d = nc.dram_tensor("wqh", [C, NH * D], F8, kind="ExternalInput").ap()
    wql_d = nc.dram_tensor("wql", [C, NH * D], F8, kind="ExternalInput").ap()
    wkh_d = nc.dram_tensor("wkh", [C, D], F8, kind="ExternalInput").ap()
    wkl_d = nc.dram_tensor("wkl", [C, D], F8, kind="ExternalInput").ap()
    wvh_d = nc.dram_tensor("wvh", [C, D], F8, kind="ExternalInput").ap()
    wvl_d = nc.dram_tensor("wvl", [C, D], F8, kind="ExternalInput").ap()
    woh_d = nc.dram_tensor("woh", [P, 2 * NJT * 2 * TCH], F8, kind="ExternalInput").ap()
    wol_d = nc.dram_tensor("wol", [P, 2 * NJT * 2 * TCH], F8, kind="ExternalInput").ap()
    cos_d = nc.dram_tensor("cosT", [P, T], BF16, kind="ExternalInput").ap()
    sin_d = nc.dram_tensor("sinT", [P, T], BF16, kind="ExternalInput").ap()
    tri_d = nc.dram_tensor("tri", [P, P], BF16, kind="ExternalInput").ap()
    on8_d = nc.dram_tensor("on8", [P, 256], F8, kind="ExternalInput").ap()
    onb_d = nc.dram_tensor("onb", [P, P], BF16, kind="ExternalInput").ap()
    y_d = nc.dram_tensor("y", [T, C], BF16, kind="ExternalOutput").ap()

    with tile.TileContext(nc) as tc, ExitStack() as ctx:
        wpool = ctx.enter_context(tc.tile_pool(name="weights", bufs=1))
        tpool = ctx.enter_context(tc.tile_pool(name="tables", bufs=1))
        trigp = ctx.enter_context(tc.tile_pool(name="trig", bufs=2))
        state = ctx.enter_context(tc.tile_pool(name="state", bufs=1))

        wqh_sb = wpool.tile([P, NKP, 2, NH * D], F8, tag="wqh")
        wql_sb = wpool.tile([P, NKP, 2, NH * D], F8, tag="wql")
        wkh_sb = wpool.tile([P, NKP, 2, D], F8, tag="wkh")
        wkl_sb = wpool.tile([P, NKP, 2, D], F8, tag="wkl")
        wvh_sb = wpool.tile([P, NKP, 2, D], F8, tag="wvh")
        wvl_sb = wpool.tile([P, NKP, 2, D], F8, tag="wvl")

        def wload(sb, dram):
            nc.scalar.dma_start(
                sb[:], dram.rearrange("(kp two p) o -> p kp two o", p=P, two=2))

        def wload_q(sb, dram):
            for c4 in range(4):
                nc.scalar.dma_start(
                    sb[:, 2 * c4:2 * c4 + 2, :, :],
                    dram[c4 * 4 * P:(c4 + 1) * 4 * P, :].rearrange(
                        "(kp two p) o -> p kp two o", p=P, two=2))

        wload(wkh_sb, wkh_d)
        wload(wvh_sb, wvh_d)
        wload_q(wqh_sb, wqh_d)
        wload(wkl_sb, wkl_d)
        wload(wvl_sb, wvl_d)
        wload_q(wql_sb, wql_d)

        tri_sb = tpool.tile([P, P], BF16, tag="tri")
        nc.gpsimd.dma_start(tri_sb[:], tri_d[:])
        on8_sb = tpool.tile([P, 2, P], F8, tag="on8")
        nc.gpsimd.dma_start(on8_sb[:], on8_d.rearrange("p (two f) -> p two f", two=2))
        onb_sb = tpool.tile([P, P], BF16, tag="onb")
        nc.gpsimd.dma_start(onb_sb[:], onb_d[:])
        identb = tpool.tile([P, P], BF16, tag="ident")
        make_identity(nc, identb[:])
        # (variant parsed below, before use)

        # PE warmup: keep the tensor engine continuously busy from t~0 so the
        # p-state ramp completes before the first real (DMA-gated) matmuls.
        warmp = ctx.enter_context(tc.tile_pool(name="warm", bufs=1))
        warm_sb = warmp.tile([P, P], BF16, tag="w")
        nc.vector.memset(warm_sb[:], 0.0)
        nwarm = 0
        for v in variant.split("+"):
            if v.startswith("warm"):
                nwarm = int(v[4:])
        with tc.tile_pool(name="psW", bufs=1, space="PSUM") as psW:
            wp = psW.tile([P, P], F32, tag="wp")
            for i in range(nwarm):
                nc.tensor.matmul(wp[:], identb[:], warm_sb[:],
                                 start=(i == 0), stop=(i == nwarm - 1))

        qrot = state.tile([P, NH, T], BF16, tag="qrot")
        krot = state.tile([P, T], BF16, tag="krot")
        v_sb = state.tile([P, NST, D], BF16, tag="v")
        v8h = state.tile([P, NST // 2, 2, D], F8, tag="v8h")
        v8l = state.tile([P, NST // 2, 2, D], F8, tag="v8l")
        ot8h = state.tile([P, 2, NST, 2, D], F8, tag="ot8h")
        ot8l = state.tile([P, 2, NST, 2, D], F8, tag="ot8l")

        njt_lim = NJT
        for v in variant.split("+"):
            if v.startswith("njt"):
                njt_lim = int(v[3:])

        # ---------------- Phase A: projections + RoPE -----------------
        with ExitStack() as actx:
          if "A" in phases:
            xpool = actx.enter_context(tc.tile_pool(name="xsub", bufs=3))
            ropep = actx.enter_context(tc.tile_pool(name="rope", bufs=2))
            vtp = actx.enter_context(tc.tile_pool(name="vt", bufs=2))
            psA = actx.enter_context(tc.tile_pool(name="psA", bufs=1, space="PSUM"))
            psT = actx.enter_context(tc.tile_pool(name="psT", bufs=2, space="PSUM"))

            def rope(acc_ps, out_ap, cos_t, sin_t):
                m1 = ropep.tile([P, TCH], F32, tag="m1")
                m2 = ropep.tile([P, TCH], F32, tag="m2")
                nc.vector.tensor_tensor(
                    m1[:], acc_ps[:], cos_t[:], mybir.AluOpType.mult)
                nc.vector.tensor_tensor(
                    m2[0:64, :], acc_ps[64:128, :], sin_t[0:64, :],
                    mybir.AluOpType.mult)
                nc.vector.tensor_tensor(
                    m2[64:128, :], acc_ps[0:64, :], sin_t[64:128, :],
                    mybir.AluOpType.mult)
                nc.vector.tensor_tensor(
                    out_ap, m1[:], m2[:], mybir.AluOpType.add)

            def w_slice(term, o, m):
                if o < NH:
                    sb = wqh_sb if term != 2 else wql_sb
                    return sb[:, m, :, o * D:(o + 1) * D]
                if o == NH:
                    sb = wkh_sb if term != 2 else wkl_sb
                    return sb[:, m, :, :]
                sb = wvh_sb if term != 2 else wvl_sb
                return sb[:, m, :, :]

            def finish(o, acc, jt, cos_t, sin_t):
                """Returns a deferred-emission thunk (or None)."""
                if "noropeA" in variant:
                    return None
                if o < NH:
                    rope(acc, qrot[:, o, jt * TCH:(jt + 1) * TCH], cos_t, sin_t)
                    return None
                if o == NH:
                    rope(acc, krot[:, jt * TCH:(jt + 1) * TCH], cos_t, sin_t)
                    return None
                vt = vtp.tile([P, TCH], BF16, tag="vt")
                nc.scalar.activation(
                    vt[:], acc[:], mybir.ActivationFunctionType.Copy, scale=SV)

                def transposes():
                    for i in range(TCH // P):
                        s_idx = jt * (TCH // P) + i
                        pst = psT.tile([P, P], BF16, tag="pst")
                        nc.tensor.transpose(pst[:], vt[:, i * P:(i + 1) * P],
                                            identb[:])
                        nc.scalar.copy(v_sb[:, s_idx, :], pst[:])
                        nc.scalar.copy(v8h[:, s_idx // 2, s_idx % 2, :], pst[:])
                        nc.vector.tensor_tensor(
                            v8l[:, s_idx // 2, s_idx % 2, :], pst[:],
                            v8h[:, s_idx // 2, s_idx % 2, :],
                            mybir.AluOpType.subtract)
                return transposes

            nacc = 0
            deferred = None
            for jt in range(njt_lim):
                cos_t = trigp.tile([P, TCH], BF16, tag="cos")
                sin_t = trigp.tile([P, TCH], BF16, tag="sin")
                ch = slice(jt * TCH, (jt + 1) * TCH)
                nc.gpsimd.dma_start(cos_t[:], cos_d[:, ch])
                nc.gpsimd.dma_start(sin_t[:], sin_d[:, ch])

                xhs, xls = [], []
                for m in range(NKP):
                    xt = xpool.tile([P, 2, TCH], F8, tag=f"xh{m}")
                    nc.sync.dma_start(
                        xt[:],
                        xh_d[2 * m * P:(2 * m + 2) * P, ch].rearrange(
                            "(two p) t -> p two t", p=P))
                    xhs.append(xt)
                for m in range(NKP):
                    xt = xpool.tile([P, 2, TCH], F8, tag=f"xl{m}")
                    nc.sync.dma_start(
                        xt[:],
                        xl_d[2 * m * P:(2 * m + 2) * P, ch].rearrange(
                            "(two p) t -> p two t", p=P))
                    xls.append(xt)

                if jt == 0:
                    accs = [psA.tile([P, TCH], F32, tag=f"acc{o}",
                                     name=f"acc{o}")
                            for o in range(6)]
                    for ti, (xs, term) in enumerate(((xhs, 0), (xls, 1),
                                                     (xhs, 2))):
                        for m in range(NKP):
                            for o in (4, 5, 0, 1, 2, 3):
                                nc.tensor.matmul(
                                    accs[o][:], w_slice(term, o, m),
                                    xs[m][:],
                                    start=(ti == 0 and m == 0),
                                    stop=(ti == 2 and m == NKP - 1),
                                    perf_mode=DR)
                    for o in (5, 4, 0, 1, 2, 3):
                        d = finish(o, accs[o], jt, cos_t, sin_t)
                        if d is not None:
                            deferred = d
                    nacc = 5
                else:
                    for o in (4, 5, 0, 1, 2, 3):
                        acc = psA.tile([P, TCH], F32, tag=f"acc{nacc % 6}",
                                       name=f"accr{nacc % 6}")
                        nacc += 1
                        first = True
                        for xs, term in (((xhs, 0), (xls, 1), (xhs, 2))):
                            for m in range(NKP):
                                nc.tensor.matmul(
                                    acc[:], w_slice(term, o, m), xs[m][:],
                                    start=first,
                                    stop=(term == 2 and m == NKP - 1),
                                    perf_mode=DR)
                                first = False
                        if deferred is not None:
                            deferred()
                            deferred = None
                        d = finish(o, acc, jt, cos_t, sin_t)
                        if d is not None:
                            deferred = d
                if jt == njt_lim - 1 and deferred is not None:
                    deferred()
                    deferred = None

        # ---------------- Phase B + C interleaved ----------------------
        with ExitStack() as bctx:
          if "B" in phases:
            e8p = bctx.enter_context(tc.tile_pool(name="e8", bufs=6))
            edp = bctx.enter_context(tc.tile_pool(name="ed", bufs=3))
            rcp = bctx.enter_context(tc.tile_pool(name="rc", bufs=3))
            wopool = bctx.enter_context(tc.tile_pool(name="wo", bufs=1))
            ypool = bctx.enter_context(tc.tile_pool(name="ysb", bufs=6))
            psS = bctx.enter_context(tc.tile_pool(name="psS", bufs=2, space="PSUM"))
            psO = bctx.enter_context(tc.tile_pool(name="psO", bufs=1, space="PSUM"))
            psD = bctx.enter_context(tc.tile_pool(name="psD", bufs=1, space="PSUM"))
            psC = bctx.enter_context(tc.tile_pool(name="psC", bufs=2, space="PSUM"))

            woh_sb = wopool.tile([P, 2, NJT, 2, TCH], F8, tag="woh")
            wol_sb = wopool.tile([P, 2, NJT, 2, TCH], F8, tag="wol")
            nc.sync.dma_start(
                woh_sb[:], woh_d.rearrange("p (i jc two t) -> p i jc two t",
                                           i=2, jc=NJT, two=2))
            nc.sync.dma_start(
                wol_sb[:], wol_d.rearrange("p (i jc two t) -> p i jc two t",
                                           i=2, jc=NJT, two=2))

            def c_group(tt, jc):
                yp = psC.tile([P, TCH], F32, tag="y")
                first = True
                for i in range(2):
                    for lhs, rhs in ((ot8h, woh_sb), (ot8l, woh_sb),
                                     (ot8h, wol_sb)):
                        nc.tensor.matmul(
                            yp[:], lhs[:, i, tt, :, :], rhs[:, i, jc, :, :],
                            start=first, stop=(i == 1 and rhs is wol_sb),
                            perf_mode=DR, skip_group_check=True)
                        first = False
                ys = ypool.tile([P, TCH], BF16, tag="ys")
                nc.vector.tensor_copy(ys[:], yp[:])
                nc.sync.dma_start(
                    y_d[tt * P:(tt + 1) * P, jc * TCH:(jc + 1) * TCH],
                    ys[:])

            do_c = "C" in phases

            cpend = []

            def emit_c(n=1):
                for _ in range(n):
                    if cpend:
                        c_group(*cpend.pop(0))

            for jt in range(NJT):
                ch = slice(jt * TCH, (jt + 1) * TCH)
                if do_c and jt >= 1:
                    cpend = [(4 * (jt - 1) + tt4, jc)
                             for tt4 in range(4) for jc in range(NJT)]
                for h in range(NH):
                    qch = qrot[:, h, ch]
                    ot_ps = psO.tile([P, TCH], F32, tag="ot")
                    dn_ps = psD.tile([P, TCH], F32, tag="dn")
                    npair = 2 * jt

                    def sc_pair(m):
                        sps = psS.tile([P, 2 * TCH], F32, tag="su")
                        for i in range(2):
                            js = 2 * m + i
                            nc.tensor.matmul(
                                sps[:, i * TCH:(i + 1) * TCH],
                                krot[:, js * P:(js + 1) * P], qch,
                                start=True, stop=True)
                        e8 = e8p.tile([P, 2, TCH], F8, tag="e8")
                        nc.scalar.activation(
                            e8[:], sps[:], mybir.ActivationFunctionType.Exp,
                            scale=SCALE)
                        return e8

                    def pv_pair(m, e8, start):
                        nc.tensor.matmul(
                            ot_ps[:], v8h[:, m, :, :], e8[:],
                            start=start, stop=False, perf_mode=DR,
                            skip_group_check=True)
                        nc.tensor.matmul(
                            ot_ps[:], v8l[:, m, :, :], e8[:],
                            start=False, stop=False, perf_mode=DR,
                            skip_group_check=True)
                        nc.tensor.matmul(
                            dn_ps[:], on8_sb[:], e8[:],
                            start=start, stop=False, perf_mode=DR,
                            skip_group_check=True)

                    pend = []
                    for m in range(npair):
                        e8 = sc_pair(m)
                        if len(pend) >= 4:
                            mm, ee = pend.pop(0)
                            pv_pair(mm, ee, mm == 0)
                        pend.append((m, e8))

                    dsup = []
                    for half in range(2):
                        sps = psS.tile([P, 2 * TCH], F32, tag="su")
                        ed = edp.tile([P, 2 * TCH], BF16, tag="ed")
                        widths = []
                        off = 0
                        for rr in range(2):
                            r = 2 * half + rr
                            w = TCH - r * P
                            js = 4 * jt + r
                            nc.tensor.matmul(
                                sps[:, off:off + w],
                                krot[:, js * P:(js + 1) * P],
                                qch[:, r * P:],
                                start=True, stop=True)
                            widths.append((r, off, w))
                            off += w
                        nc.scalar.activation(
                            ed[:, 0:off], sps[:, 0:off],
                            mybir.ActivationFunctionType.Exp, scale=SCALE)
                        for r, off_, w in widths:
                            nc.vector.tensor_tensor(
                                ed[:, off_:off_ + P], ed[:, off_:off_ + P],
                                tri_sb[:], mybir.AluOpType.mult)
                        dsup.append((ed, widths))
                        while pend:
                            mm, ee = pend.pop(0)
                            pv_pair(mm, ee, mm == 0)

                    for ed, widths in dsup:
                        for r, off, w in widths:
                            js = 4 * jt + r
                            nc.tensor.matmul(
                                ot_ps[:, r * P:], v_sb[:, js, :],
                                ed[:, off:off + w],
                                start=(jt == 0 and r == 0), stop=(r == 3),
                                skip_group_check=True)
                            nc.tensor.matmul(
                                dn_ps[:, r * P:], onb_sb[:],
                                ed[:, off:off + w],
                                start=(jt == 0 and r == 0), stop=(r == 3),
                                skip_group_check=True)

                    emit_c(4)
                    rb = rcp.tile([P, TCH], F32, tag="rb")
                    nc.vector.reciprocal(rb[:], dn_ps[:])
                    tmp = rcp.tile([P, 4, P], F32, tag="tmp")
                    nc.vector.tensor_tensor(
                        tmp[:].rearrange("p a b -> p (a b)"), ot_ps[:], rb[:],
                        mybir.AluOpType.mult)
                    oh = ot8h[:, h // 2, 4 * jt:4 * jt + 4, h % 2, :]
                    ol = ot8l[:, h // 2, 4 * jt:4 * jt + 4, h % 2, :]
                    nc.vector.tensor_copy(oh, tmp[:])
                    nc.vector.tensor_tensor(
                        ol, tmp[:], oh, mybir.AluOpType.subtract)


            if do_c:
                for tt4 in range(4):
                    for jc in range(NJT):
                        c_group(12 + tt4, jc)

    nc.compile()
    return nc


def host_prep(x, wq, wk, wv, wo):
    import ml_dtypes
    F8np = ml_dtypes.float8_e4m3
    BFnp = ml_dtypes.bfloat16

    x = np.asarray(x, dtype=np.float32)
    wq = np.asarray(wq, dtype=np.float32)
    wk = np.asarray(wk, dtype=np.float32)
    wv = np.asarray(wv, dtype=np.float32)
    wo = np.asarray(wo, dtype=np.float32)

    perm = np.concatenate([np.arange(0, D, 2), np.arange(1, D, 2)])

    inv_freq = (1.0 / THETA ** (np.arange(0, D, 2, dtype=np.float32) / D)).astype(np.float32)
    pos = np.arange(T, dtype=np.float32)
    freqs = pos[:, None] * inv_freq[None, :]
    cos_t = np.cos(freqs).astype(np.float32).T
    sin_t = np.sin(freqs).astype(np.float32).T
    dq = np.float32(1.0 / (SX * SW))
    cosT = np.concatenate([cos_t, cos_t], axis=0) * dq
    sinT = np.concatenate([-sin_t, sin_t], axis=0) * dq

    tri = (np.arange(P)[None, :] >= np.arange(P)[:, None]).astype(BFnp)

    def hilo(a):
        h = a.astype(F8np)
        l = (a - h.astype(np.float32)).astype(F8np)
        return h, l

    xs = [np.ascontiguousarray(x[b].T) * SX for b in range(B)]
    xhl = [hilo(a) for a in xs]

    in_maps = []
    for c in range(N_CORES):
        b, g = divmod(c, GROUP)
        rows = []
        for hh in range(NH):
            h = g * GROUP + hh
            rows.append(wq[h * D + perm, :])
        wq_g = np.concatenate(rows, axis=0) * SW          # [512, C]
        wk_g = wk[g * D + perm, :] * SW
        wv_g = wv[g * D:(g + 1) * D, :] * SW
        wo_g = wo[:, g * NH * D:(g + 1) * NH * D]         # [C, 512]

        wqh, wql = hilo(np.ascontiguousarray(wq_g.T))
        wkh, wkl = hilo(np.ascontiguousarray(wk_g.T))
        wvh, wvl = hilo(np.ascontiguousarray(wv_g.T))
        # wo pair-contiguous DR layout: [d, hpair, jc, h-in-pair, tc]
        woT = np.ascontiguousarray(wo_g.T) * SWO          # [512, C]
        wo_a = woT.reshape(2, 2, D, NJT, TCH)             # [i, hh, d, jc, tc]
        wo_b = np.ascontiguousarray(np.transpose(wo_a, (2, 0, 3, 1, 4)))
        woh, wol = hilo(wo_b.reshape(P, -1))

        in_maps.append({
            "xh": xhl[b][0], "xl": xhl[b][1],
            "wqh": wqh, "wql": wql,
            "wkh": wkh, "wkl": wkl,
            "wvh": wvh, "wvl": wvl,
            "woh": woh, "wol": wol,
            "cosT": cosT.astype(BFnp),
            "sinT": sinT.astype(BFnp),
            "tri": tri,
            "on8": np.full((P, 256), CDEN, dtype=F8np),
            "onb": np.full((P, P), CDEN, dtype=BFnp),
        })
    return in_maps


_CACHE = {}


def _get_program(key="v2"):
    if key not in _CACHE:
        _CACHE[key] = build_program()
    return _CACHE[key]


def kernel(x, mask, wq, wk, wv, wo):
    nc = _get_program()
    in_maps = host_prep(x, wq, wk, wv, wo)
    res = run_bass_kernel_spmd(nc, in_maps, list(range(N_CORES))).results
    out = np.zeros((B, T, C), dtype=np.float32)
    for c in range(N_CORES):
        out[c // GROUP] += res[c]["y"].astype(np.float32) * YDQ
    return out


# revision 6
# speedup vs baseline: 1.4026x; 1.0150x over previous
"""Causal GQA attention (B=2, T=2048, C=2048, H=16, HKV=4, D=128, RoPE)
on 8 Trainium2 NeuronCores — v2.

Sharding: core c = (batch b = c//4, kv-group g = c%4): 4 q heads + 1 kv
head per core; row-parallel output projection, host sums 4 partials.

Design notes:
  - Projections run as fp8e4 hi/lo DoubleRow matmuls (3 compensation
    terms over k-tile pairs). x and w ship pre-split/pre-scaled; dequant
    folds into the RoPE tables (q/k), the v-copy scale, and the softmax
    scale applied at exp time.
  - bf16 storage elsewhere; fp8 only where noise is attenuated
    (off-diagonal softmax weights + hi/lo v).
  - Attention is block-causal at 128 granularity via suffix-truncated
    moving operands on the diagonal; off-diagonal s-tile pairs exp
    straight to fp8 and feed DoubleRow PV (v hi/lo) and DoubleRow
    ones-matmul denominators.
  - Output projection (bf16) is interleaved into phase B's instruction
    stream to fill the tensor engine during Act-bound stretches.
  - DMA pacing: wo deferred to phase B, trig tables per-chunk bf16 with
    pool-rotation gating, x triple-buffered.
"""

import os
from contextlib import ExitStack

import numpy as np

import concourse.bass as bass
import concourse.tile as tile
from concourse import bacc, mybir
from concourse.bass_utils import run_bass_kernel_spmd
from concourse.masks import make_identity

B, T, C = 2, 2048, 2048
H, HKV, D = 16, 4, 128
GROUP = H // HKV
THETA = 1000000.0
SCALE = D ** -0.5

P = 128
TCH = 512
NJT = T // TCH             # 4
NK = C // P                # 16 k-tiles
NKP = NK // 2              # 8 DoubleRow k-tile pairs
NH = GROUP                 # 4 local q heads
NST = T // P               # 16 s-tiles
N_CORES = 8

SX = 16.0                  # x fp8 scale
SW = 1024.0                # w fp8 scale (q/k/v)
SV = 2.0 ** -10            # v psum -> sbuf scale
CDEN = 0.25                # ones constant: ot = 64*O_true (fp8 range)
SWO = 1024.0               # wo fp8 scale
YDQ = 1.0 / (64.0 * SWO)   # host-side dequant of the y partials

F32 = mybir.dt.float32
BF16 = mybir.dt.bfloat16
F8 = mybir.dt.float8e4
DR = mybir.MatmulPerfMode.DoubleRow


def build_program(phases="ABC", variant=""):
    nc = bacc.Bacc("TRN2", target_bir_lowering=False, debug=False)

    xh_d = nc.dram_tensor("xh", [C, T], F8, kind="ExternalInput").ap()
    xl_d = nc.dram_tensor("xl", [C, T], F8, kind="ExternalInput").ap()
    WPK = NH * D + 2 * D       # packed row: wq 512 | wk 128 | wv 128
    wAh_d = nc.dram_tensor("wAh", [P, NKP * 2 * WPK], F8, kind="ExternalInput").ap()
    wAl_d = nc.dram_tensor("wAl", [P, NKP * 2 * WPK], F8, kind="ExternalInput").ap()
    woh_d = nc.dram_tensor("woh", [P, 2 * NJT * 2 * TCH], F8, kind="ExternalInput").ap()
    wol_d = nc.dram_tensor("wol", [P, 2 * NJT * 2 * TCH], F8, kind="ExternalInput").ap()
    cos_d = nc.dram_tensor("cosT", [P, T], BF16, kind="ExternalInput").ap()
    sin_d = nc.dram_tensor("sinT", [P, T], BF16, kind="ExternalInput").ap()
    tri_d = nc.dram_tensor("tri", [P, P], BF16, kind="ExternalInput").ap()
    on8_d = nc.dram_tensor("on8", [P, 256], F8, kind="ExternalInput").ap()
    onb_d = nc.dram_tensor("onb", [P, P], BF16, kind="ExternalInput").ap()
    y_d = nc.dram_tensor("y", [T, C], BF16, kind="ExternalOutput").ap()

    with tile.TileContext(nc) as tc, ExitStack() as ctx:
        wpool = ctx.enter_context(tc.tile_pool(name="weights", bufs=1))
        tpool = ctx.enter_context(tc.tile_pool(name="tables", bufs=1))
        trigp = ctx.enter_context(tc.tile_pool(name="trig", bufs=2))
        state = ctx.enter_context(tc.tile_pool(name="state", bufs=1))

        wAh_sb = wpool.tile([P, NKP, 2, WPK], F8, tag="wAh")
        wAl_sb = wpool.tile([P, NKP, 2, WPK], F8, tag="wAl")
        for h4 in range(4):
            nc.scalar.dma_start(
                wAh_sb[:, 2 * h4:2 * h4 + 2, :, :],
                wAh_d[:, 2 * h4 * 2 * WPK:(2 * h4 + 2) * 2 * WPK].rearrange(
                    "p (kp two o) -> p kp two o", kp=2, two=2))
        for h4 in range(4):
            nc.scalar.dma_start(
                wAl_sb[:, 2 * h4:2 * h4 + 2, :, :],
                wAl_d[:, 2 * h4 * 2 * WPK:(2 * h4 + 2) * 2 * WPK].rearrange(
                    "p (kp two o) -> p kp two o", kp=2, two=2))

        tri_sb = tpool.tile([P, P], BF16, tag="tri")
        nc.gpsimd.dma_start(tri_sb[:], tri_d[:])
        on8_sb = tpool.tile([P, 2, P], F8, tag="on8")
        nc.gpsimd.dma_start(on8_sb[:], on8_d.rearrange("p (two f) -> p two f", two=2))
        onb_sb = tpool.tile([P, P], BF16, tag="onb")
        nc.gpsimd.dma_start(onb_sb[:], onb_d[:])
        identb = tpool.tile([P, P], BF16, tag="ident")
        make_identity(nc, identb[:])
        # (variant parsed below, before use)

        # PE warmup: keep the tensor engine continuously busy from t~0 so the
        # p-state ramp completes before the first real (DMA-gated) matmuls.
        warmp = ctx.enter_context(tc.tile_pool(name="warm", bufs=1))
        warm_sb = warmp.tile([P, P], BF16, tag="w")
        nc.vector.memset(warm_sb[:], 0.0)
        nwarm = 0
        for v in variant.split("+"):
            if v.startswith("warm"):
                nwarm = int(v[4:])
        with tc.tile_pool(name="psW", bufs=1, space="PSUM") as psW:
            wp = psW.tile([P, P], F32, tag="wp")
            for i in range(nwarm):
                nc.tensor.matmul(wp[:], identb[:], warm_sb[:],
                                 start=(i == 0), stop=(i == nwarm - 1))

        qrot = state.tile([P, NH, T], BF16, tag="qrot")
        krot = state.tile([P, T], BF16, tag="krot")
        v_sb = state.tile([P, NST, D], BF16, tag="v")
        v8h = state.tile([P, NST // 2, 2, D], F8, tag="v8h")
        v8l = state.tile([P, NST // 2, 2, D], F8, tag="v8l")
        ot8h = state.tile([P, 2, NST, 2, D], F8, tag="ot8h")
        ot8l = state.tile([P, 2, NST, 2, D], F8, tag="ot8l")

        njt_lim = NJT
        for v in variant.split("+"):
            if v.startswith("njt"):
                njt_lim = int(v[3:])

        # ---------------- Phase A: projections + RoPE -----------------
        with ExitStack() as actx:
          if "A" in phases:
            xpool = actx.enter_context(tc.tile_pool(name="xsub", bufs=3))
            ropep = actx.enter_context(tc.tile_pool(name="rope", bufs=2))
            vtp = actx.enter_context(tc.tile_pool(name="vt", bufs=2))
            psA = actx.enter_context(tc.tile_pool(name="psA", bufs=1, space="PSUM"))
            psT = actx.enter_context(tc.tile_pool(name="psT", bufs=2, space="PSUM"))

            def rope(acc_ps, out_ap, cos_t, sin_t):
                m1 = ropep.tile([P, TCH], F32, tag="m1")
                m2 = ropep.tile([P, TCH], F32, tag="m2")
                nc.vector.tensor_tensor(
                    m1[:], acc_ps[:], cos_t[:], mybir.AluOpType.mult)
                nc.vector.tensor_tensor(
                    m2[0:64, :], acc_ps[64:128, :], sin_t[0:64, :],
                    mybir.AluOpType.mult)
                nc.vector.tensor_tensor(
                    m2[64:128, :], acc_ps[0:64, :], sin_t[64:128, :],
                    mybir.AluOpType.mult)
                nc.vector.tensor_tensor(
                    out_ap, m1[:], m2[:], mybir.AluOpType.add)

            def w_slice(term, o, m):
                sb = wAh_sb if term != 2 else wAl_sb
                if o < NH:
                    return sb[:, m, :, o * D:(o + 1) * D]
                if o == NH:
                    return sb[:, m, :, NH * D:NH * D + D]
                return sb[:, m, :, NH * D + D:NH * D + 2 * D]

            def finish(o, acc, jt, cos_t, sin_t):
                """Returns a deferred-emission thunk (or None)."""
                if "noropeA" in variant:
                    return None
                if o < NH:
                    rope(acc, qrot[:, o, jt * TCH:(jt + 1) * TCH], cos_t, sin_t)
                    return None
                if o == NH:
                    rope(acc, krot[:, jt * TCH:(jt + 1) * TCH], cos_t, sin_t)
                    return None
                vt = vtp.tile([P, TCH], BF16, tag="vt")
                nc.scalar.activation(
                    vt[:], acc[:], mybir.ActivationFunctionType.Copy, scale=SV)

                def transposes():
                    for i in range(TCH // P):
                        s_idx = jt * (TCH // P) + i
                        pst = psT.tile([P, P], BF16, tag="pst")
                        nc.tensor.transpose(pst[:], vt[:, i * P:(i + 1) * P],
                                            identb[:])
                        nc.scalar.copy(v_sb[:, s_idx, :], pst[:])
                        nc.scalar.copy(v8h[:, s_idx // 2, s_idx % 2, :], pst[:])
                        nc.vector.tensor_tensor(
                            v8l[:, s_idx // 2, s_idx % 2, :], pst[:],
                            v8h[:, s_idx // 2, s_idx % 2, :],
                            mybir.AluOpType.subtract)
                return transposes

            nacc = 0
            deferred = None
            for jt in range(njt_lim):
                cos_t = trigp.tile([P, TCH], BF16, tag="cos")
                sin_t = trigp.tile([P, TCH], BF16, tag="sin")
                ch = slice(jt * TCH, (jt + 1) * TCH)
                nc.gpsimd.dma_start(cos_t[:], cos_d[:, ch])
                nc.gpsimd.dma_start(sin_t[:], sin_d[:, ch])

                xhs, xls = [], []
                for m in range(NKP):
                    xt = xpool.tile([P, 2, TCH], F8, tag=f"xh{m}")
                    nc.sync.dma_start(
                        xt[:],
                        xh_d[2 * m * P:(2 * m + 2) * P, ch].rearrange(
                            "(two p) t -> p two t", p=P))
                    xhs.append(xt)
                for m in range(NKP):
                    xt = xpool.tile([P, 2, TCH], F8, tag=f"xl{m}")
                    nc.sync.dma_start(
                        xt[:],
                        xl_d[2 * m * P:(2 * m + 2) * P, ch].rearrange(
                            "(two p) t -> p two t", p=P))
                    xls.append(xt)

                if jt == 0:
                    accs = [psA.tile([P, TCH], F32, tag=f"acc{o}",
                                     name=f"acc{o}")
                            for o in range(6)]
                    for ti, (xs, term) in enumerate(((xhs, 0), (xls, 1),
                                                     (xhs, 2))):
                        for m in range(NKP):
                            for o in (4, 5, 0, 1, 2, 3):
                                nc.tensor.matmul(
                                    accs[o][:], w_slice(term, o, m),
                                    xs[m][:],
                                    start=(ti == 0 and m == 0),
                                    stop=(ti == 2 and m == NKP - 1),
                                    perf_mode=DR)
                    for o in (5, 4, 0, 1, 2, 3):
                        d = finish(o, accs[o], jt, cos_t, sin_t)
                        if d is not None:
                            deferred = d
                    nacc = 5
                else:
                    for o in (4, 5, 0, 1, 2, 3):
                        acc = psA.tile([P, TCH], F32, tag=f"acc{nacc % 6}",
                                       name=f"accr{nacc % 6}")
                        nacc += 1
                        first = True
                        for xs, term in (((xhs, 0), (xls, 1), (xhs, 2))):
                            for m in range(NKP):
                                nc.tensor.matmul(
                                    acc[:], w_slice(term, o, m), xs[m][:],
                                    start=first,
                                    stop=(term == 2 and m == NKP - 1),
                                    perf_mode=DR)
                                first = False
                        if deferred is not None:
                            deferred()
                            deferred = None
                        d = finish(o, acc, jt, cos_t, sin_t)
                        if d is not None:
                            deferred = d
                if jt == njt_lim - 1 and deferred is not None:
                    deferred()
                    deferred = None

        # ---------------- Phase B + C interleaved ----------------------
        with ExitStack() as bctx:
          if "B" in phases:
            e8p = bctx.enter_context(tc.tile_pool(name="e8", bufs=6))
            edp = bctx.enter_context(tc.tile_pool(name="ed", bufs=3))
            rcp = bctx.enter_context(tc.tile_pool(name="rc", bufs=3))
            wopool = bctx.enter_context(tc.tile_pool(name="wo", bufs=1))
            ypool = bctx.enter_context(tc.tile_pool(name="ysb", bufs=6))
            psS = bctx.enter_context(tc.tile_pool(name="psS", bufs=2, space="PSUM"))
            psO = bctx.enter_context(tc.tile_pool(name="psO", bufs=1, space="PSUM"))
            psD = bctx.enter_context(tc.tile_pool(name="psD", bufs=1, space="PSUM"))
            psC = bctx.enter_context(tc.tile_pool(name="psC", bufs=2, space="PSUM"))

            woh_sb = wopool.tile([P, 2, NJT, 2, TCH], F8, tag="woh")
            wol_sb = wopool.tile([P, 2, NJT, 2, TCH], F8, tag="wol")
            nc.sync.dma_start(
                woh_sb[:], woh_d.rearrange("p (i jc two t) -> p i jc two t",
                                           i=2, jc=NJT, two=2))
            nc.sync.dma_start(
                wol_sb[:], wol_d.rearrange("p (i jc two t) -> p i jc two t",
                                           i=2, jc=NJT, two=2))

            def c_group(tt, jc):
                yp = psC.tile([P, TCH], F32, tag="y")
                first = True
                for i in range(2):
                    for lhs, rhs in ((ot8h, woh_sb), (ot8l, woh_sb),
                                     (ot8h, wol_sb)):
                        nc.tensor.matmul(
                            yp[:], lhs[:, i, tt, :, :], rhs[:, i, jc, :, :],
                            start=first, stop=(i == 1 and rhs is wol_sb),
                            perf_mode=DR, skip_group_check=True)
                        first = False
                ys = ypool.tile([P, TCH], BF16, tag="ys")
                nc.vector.tensor_copy(ys[:], yp[:])
                nc.sync.dma_start(
                    y_d[tt * P:(tt + 1) * P, jc * TCH:(jc + 1) * TCH],
                    ys[:])

            do_c = "C" in phases

            cpend = []

            def emit_c(n=1):
                for _ in range(n):
                    if cpend:
                        c_group(*cpend.pop(0))

            for jt in range(NJT):
                ch = slice(jt * TCH, (jt + 1) * TCH)
                if do_c and jt >= 1:
                    cpend = [(4 * (jt - 1) + tt4, jc)
                             for tt4 in range(4) for jc in range(NJT)]
                for h in range(NH):
                    qch = qrot[:, h, ch]
                    ot_ps = psO.tile([P, TCH], F32, tag="ot")
                    dn_ps = psD.tile([P, TCH], F32, tag="dn")
                    npair = 2 * jt

                    def sc_pair(m):
                        sps = psS.tile([P, 2 * TCH], F32, tag="su")
                        for i in range(2):
                            js = 2 * m + i
                            nc.tensor.matmul(
                                sps[:, i * TCH:(i + 1) * TCH],
                                krot[:, js * P:(js + 1) * P], qch,
                                start=True, stop=True)
                        e8 = e8p.tile([P, 2, TCH], F8, tag="e8")
                        nc.scalar.activation(
                            e8[:], sps[:], mybir.ActivationFunctionType.Exp,
                            scale=SCALE)
                        return e8

                    def pv_pair(m, e8, start):
                        nc.tensor.matmul(
                            ot_ps[:], v8h[:, m, :, :], e8[:],
                            start=start, stop=False, perf_mode=DR,
                            skip_group_check=True)
                        nc.tensor.matmul(
                            ot_ps[:], v8l[:, m, :, :], e8[:],
                            start=False, stop=False, perf_mode=DR,
                            skip_group_check=True)
                        nc.tensor.matmul(
                            dn_ps[:], on8_sb[:], e8[:],
                            start=start, stop=False, perf_mode=DR,
                            skip_group_check=True)

                    pend = []
                    for m in range(npair):
                        e8 = sc_pair(m)
                        if len(pend) >= 4:
                            mm, ee = pend.pop(0)
                            pv_pair(mm, ee, mm == 0)
                        pend.append((m, e8))

                    dsup = []
                    for half in range(2):
                        sps = psS.tile([P, 2 * TCH], F32, tag="su")
                        ed = edp.tile([P, 2 * TCH], BF16, tag="ed")
                        widths = []
                        off = 0
                        for rr in range(2):
                            r = 2 * half + rr
                            w = TCH - r * P
                            js = 4 * jt + r
                            nc.tensor.matmul(
                                sps[:, off:off + w],
                                krot[:, js * P:(js + 1) * P],
                                qch[:, r * P:],
                                start=True, stop=True)
                            widths.append((r, off, w))
                            off += w
                        nc.scalar.activation(
                            ed[:, 0:off], sps[:, 0:off],
                            mybir.ActivationFunctionType.Exp, scale=SCALE)
                        for r, off_, w in widths:
                            nc.vector.tensor_tensor(
                                ed[:, off_:off_ + P], ed[:, off_:off_ + P],
                                tri_sb[:], mybir.AluOpType.mult)
                        dsup.append((ed, widths))
                        while pend:
                            mm, ee = pend.pop(0)
                            pv_pair(mm, ee, mm == 0)

                    for ed, widths in dsup:
                        for r, off, w in widths:
                            js = 4 * jt + r
                            nc.tensor.matmul(
                                ot_ps[:, r * P:], v_sb[:, js, :],
                                ed[:, off:off + w],
                                start=(jt == 0 and r == 0), stop=(r == 3),
                                skip_group_check=True)
                            nc.tensor.matmul(
                                dn_ps[:, r * P:], onb_sb[:],
                                ed[:, off:off + w],
                                start=(jt == 0 and r == 0), stop=(r == 3),
                                skip_group_check=True)

                    emit_c(4)
                    rb = rcp.tile([P, TCH], F32, tag="rb")
                    nc.vector.reciprocal(rb[:], dn_ps[:])
                    tmp = rcp.tile([P, 4, P], F32, tag="tmp")
                    nc.vector.tensor_tensor(
                        tmp[:].rearrange("p a b -> p (a b)"), ot_ps[:], rb[:],
                        mybir.AluOpType.mult)
                    oh = ot8h[:, h // 2, 4 * jt:4 * jt + 4, h % 2, :]
                    ol = ot8l[:, h // 2, 4 * jt:4 * jt + 4, h % 2, :]
                    nc.vector.tensor_copy(oh, tmp[:])
                    nc.vector.tensor_tensor(
                        ol, tmp[:], oh, mybir.AluOpType.subtract)


            if do_c:
                for tt4 in range(4):
                    for jc in range(NJT):
                        c_group(12 + tt4, jc)

    nc.compile()
    return nc


def host_prep(x, wq, wk, wv, wo):
    import ml_dtypes
    F8np = ml_dtypes.float8_e4m3
    BFnp = ml_dtypes.bfloat16

    x = np.asarray(x, dtype=np.float32)
    wq = np.asarray(wq, dtype=np.float32)
    wk = np.asarray(wk, dtype=np.float32)
    wv = np.asarray(wv, dtype=np.float32)
    wo = np.asarray(wo, dtype=np.float32)

    perm = np.concatenate([np.arange(0, D, 2), np.arange(1, D, 2)])

    inv_freq = (1.0 / THETA ** (np.arange(0, D, 2, dtype=np.float32) / D)).astype(np.float32)
    pos = np.arange(T, dtype=np.float32)
    freqs = pos[:, None] * inv_freq[None, :]
    cos_t = np.cos(freqs).astype(np.float32).T
    sin_t = np.sin(freqs).astype(np.float32).T
    dq = np.float32(1.0 / (SX * SW))
    cosT = np.concatenate([cos_t, cos_t], axis=0) * dq
    sinT = np.concatenate([-sin_t, sin_t], axis=0) * dq

    tri = (np.arange(P)[None, :] >= np.arange(P)[:, None]).astype(BFnp)

    def hilo(a):
        h = a.astype(F8np)
        l = (a - h.astype(np.float32)).astype(F8np)
        return h, l

    xs = [np.ascontiguousarray(x[b].T) * SX for b in range(B)]
    xhl = [hilo(a) for a in xs]

    in_maps = []
    for c in range(N_CORES):
        b, g = divmod(c, GROUP)
        rows = []
        for hh in range(NH):
            h = g * GROUP + hh
            rows.append(wq[h * D + perm, :])
        wq_g = np.concatenate(rows, axis=0) * SW          # [512, C]
        wk_g = wk[g * D + perm, :] * SW
        wv_g = wv[g * D:(g + 1) * D, :] * SW
        wo_g = wo[:, g * NH * D:(g + 1) * NH * D]         # [C, 512]

        wqT = np.ascontiguousarray(wq_g.T)                # [C, 512]
        wkT = np.ascontiguousarray(wk_g.T)                # [C, 128]
        wvT = np.ascontiguousarray(wv_g.T)                # [C, 128]
        rows = np.arange(C).reshape(NKP, 2, P)            # [kp, two, p]
        pk = np.concatenate([wqT[rows], wkT[rows], wvT[rows]], axis=-1)
        pk = np.ascontiguousarray(np.transpose(pk, (2, 0, 1, 3)))  # [p,kp,two,col]
        wAh, wAl = hilo(pk.reshape(P, -1))
        # wo pair-contiguous DR layout: [d, hpair, jc, h-in-pair, tc]
        woT = np.ascontiguousarray(wo_g.T) * SWO          # [512, C]
        wo_a = woT.reshape(2, 2, D, NJT, TCH)             # [i, hh, d, jc, tc]
        wo_b = np.ascontiguousarray(np.transpose(wo_a, (2, 0, 3, 1, 4)))
        woh, wol = hilo(wo_b.reshape(P, -1))

        in_maps.append({
            "xh": xhl[b][0], "xl": xhl[b][1],
            "wAh": wAh, "wAl": wAl,
            "woh": woh, "wol": wol,
            "cosT": cosT.astype(BFnp),
            "sinT": sinT.astype(BFnp),
            "tri": tri,
            "on8": np.full((P, 256), CDEN, dtype=F8np),
            "onb": np.full((P, P), CDEN, dtype=BFnp),
        })
    return in_maps


_CACHE = {}


def _get_program(key="v2"):
    if key not in _CACHE:
        _CACHE[key] = build_program()
    return _CACHE[key]


def kernel(x, mask, wq, wk, wv, wo):
    nc = _get_program()
    in_maps = host_prep(x, wq, wk, wv, wo)
    res = run_bass_kernel_spmd(nc, in_maps, list(range(N_CORES))).results
    out = np.zeros((B, T, C), dtype=np.float32)
    for c in range(N_CORES):
        out[c // GROUP] += res[c]["y"].astype(np.float32) * YDQ
    return out
